# revision 1
# baseline (speedup 1.0000x reference)
"""Trainium2 Bass kernel for nn_CSA_ConvBlock (conv-self-attention block).

Reference math (B,C,H,W = 16,256,64,64):
  fq = conv3x3(x, wq); fk = conv3x3(x, wk); fv = conv3x3(x, wv)
  q_sum = fq.sum(H); k_sum = fk.sum(C,H)
  f_scores[b,c] = sum_w q_sum[b,c,w]*k_sum[b,w] / (sqrt(W)*H^2)
  scores = softmax_C(f_scores)
  out = relu(BN_eval(scores*fv + x))

Key algebraic reduction: fq and fk are only consumed through H-sums, and
conv is linear, so q_sum/k_sum collapse to 3-tap-x-3-dy matmuls over the
column sums of x (with top/bottom row edge corrections for SAME padding).
Only conv(x, wv) is computed in full.  Since scores ~ 1/C ~ 0.004, the
attention branch is strongly suppressed relative to the fp32 residual x,
so bf16 matmuls are numerically safe.

Sharding: data-parallel over batch, 2 batches per core on 8 cores.
"""

import os
import sys
import numpy as np
from contextlib import ExitStack

if "/opt/trn_rl_repo" not in sys.path and not any(
    "trn_rl_repo" in p for p in sys.path
):
    sys.path.insert(0, "/opt/trn_rl_repo")

import concourse.bass as bass
import concourse.tile as tile
from concourse import bacc, mybir
from concourse import bass_utils

B, C, H, W = 16, 256, 64, 64
NCORES = 8
BPC = B // NCORES          # batches per core
P = 128                    # partitions
KT = C // P                # channel k-tiles (2)
MT = C // P                # channel m-tiles (2)
PW = W + 2                 # padded width 66
PH = H + 2                 # padded height 66
NTAP = 9
ROWS_PER_ST = 8
NF = ROWS_PER_ST * W       # 512 free elems per spatial tile
ST = (H * W) // NF         # 8 spatial tiles per (batch, mtile)
EPS = 1e-5
SCORE_SCALE = 1.0 / (np.sqrt(np.float32(W)) * (H * H))  # 1/32768

FP32 = mybir.dt.float32
BF16 = mybir.dt.bfloat16
AX = mybir.AxisListType
ALU = mybir.AluOpType
ACTF = mybir.ActivationFunctionType


def _emit(ctx: ExitStack, tc: "tile.TileContext", nc, x, wqT_d, wvT_d,
          wks_d, inv_d, bias2_d, out, parts=("scores", "conv", "fullevict"),
          prep_state=None):
    if prep_state is None:
        prep_state = _emit_prep(ctx, tc, nc, wqT_d, wvT_d, wks_d, inv_d,
                                bias2_d)
    consts, wqT, wvT, wks, ones_col, inv_t, bias2_t = prep_state
    return _emit_main(ctx, tc, nc, x, out, parts, consts, wqT, wvT, wks,
                      ones_col, inv_t, bias2_t)


def _emit_prep(ctx, tc, nc, wqT_d, wvT_d, wks_d, inv_d, bias2_d):
    """Weights arrive pre-transposed/cast from the host; just stage them."""
    consts = ctx.enter_context(tc.tile_pool(name="consts", bufs=1))
    ones_col = consts.tile([P, 1], FP32, tag="ones")
    nc.vector.memset(ones_col[:], 1.0)

    inv_t, bias2_t = [], []
    for mt in range(MT):
        iv = consts.tile([P, 1], FP32, tag=f"inv{mt}")
        nc.sync.dma_start(iv[:], inv_d[mt * P:(mt + 1) * P])
        inv_t.append(iv)
        b2 = consts.tile([P, 1], FP32, tag=f"b2{mt}")
        nc.sync.dma_start(b2[:], bias2_d[mt * P:(mt + 1) * P])
        bias2_t.append(b2)

    wT_pool = ctx.enter_context(tc.tile_pool(name="wT", bufs=1))
    wqT = [wT_pool.tile([P, NTAP * C], BF16, tag=f"wqT{kt}", name=f"wqT{kt}")
           for kt in range(KT)]
    wvT = [wT_pool.tile([P, NTAP * C], BF16, tag=f"wvT{kt}", name=f"wvT{kt}")
           for kt in range(KT)]
    wks = [wT_pool.tile([P, NTAP], BF16, tag=f"wks{kt}", name=f"wks{kt}")
           for kt in range(KT)]
    for kt in range(KT):
        nc.sync.dma_start(wvT[kt][:], wvT_d[kt])
        nc.sync.dma_start(wqT[kt][:], wqT_d[kt])
        nc.sync.dma_start(wks[kt][:], wks_d[kt])

    return (consts, wqT, wvT, wks, ones_col, inv_t, bias2_t)


def _emit_main(ctx, tc, nc, x, out, parts, consts, wqT, wvT, wks,
               ones_col, inv_t, bias2_t):
    # ---- main per-batch pipeline ----
    xf32_pool = ctx.enter_context(tc.tile_pool(name="xf32", bufs=2 * KT))
    xpad_pool = ctx.enter_context(tc.tile_pool(name="xpad", bufs=2 * KT))
    agg_pool = ctx.enter_context(tc.tile_pool(name="agg", bufs=2 * KT))
    small = ctx.enter_context(tc.tile_pool(name="small", bufs=2))
    ev_pool = ctx.enter_context(tc.tile_pool(name="ev", bufs=3))
    qk_psum = ctx.enter_context(tc.tile_pool(name="qk_psum", bufs=1, space="PSUM"))
    misc_psum = ctx.enter_context(tc.tile_pool(name="misc_psum", bufs=1, space="PSUM"))
    fv_psum = ctx.enter_context(tc.tile_pool(name="fv_psum", bufs=5, space="PSUM"))

    for b in range(BPC):
        # load x (fp32, flat) per k-tile
        xf = []
        for kt in range(KT):
            t = xf32_pool.tile([P, H * W], FP32, tag="xf32")
            nc.sync.dma_start(t[:], x[b, kt * P:(kt + 1) * P])
            xf.append(t)

        # padded bf16 image + column-sum aggregates
        xp, aggs = [], []
        for kt in range(KT):
            tp = xpad_pool.tile([P, PH * PW], BF16, tag="xpad")
            t3 = tp[:].rearrange("p (r c) -> p r c", c=PW)
            # zero only the 1-wide borders (top/bottom rows, left/right cols)
            nc.vector.memset(t3[:, 0, :], 0.0)
            nc.vector.memset(t3[:, PH - 1, :], 0.0)
            nc.vector.memset(t3[:, 1:PH - 1, 0], 0.0)
            nc.vector.memset(t3[:, 1:PH - 1, PW - 1], 0.0)
            nc.vector.tensor_copy(
                t3[:, 1:H + 1, 1:W + 1],
                xf[kt][:].rearrange("p (h w) -> p h w", w=W))
            xp.append(tp)

            if "scores" not in parts:
                continue
            cs = small.tile([P, W], FP32, tag="cs")
            nc.vector.tensor_reduce(
                cs[:], xf[kt][:].rearrange("p (h w) -> p w h", w=W),
                axis=AX.X, op=ALU.add)
            xhw = xf[kt][:].rearrange("p (h w) -> p h w", w=W)
            ag = agg_pool.tile([P, 3 * PW], BF16, tag="agg")
            a3 = ag[:].rearrange("p (a c) -> p a c", c=PW)
            nc.vector.memset(a3[:, :, 0], 0.0)
            nc.vector.memset(a3[:, :, PW - 1], 0.0)
            # dy=0 row-window is rows -1..H-2: colsum - bottom row
            nc.vector.tensor_sub(a3[:, 0, 1:W + 1], cs[:], xhw[:, H - 1, :])
            nc.vector.tensor_copy(a3[:, 1, 1:W + 1], cs[:])
            # dy=2 row-window is rows 1..H: colsum - top row
            nc.vector.tensor_sub(a3[:, 2, 1:W + 1], cs[:], xhw[:, 0, :])
            aggs.append(ag)

        if "scores" not in parts:
            s1 = inv_t
        else:
            # Transposed layout: qT[w, c] and kT[w, 1] accumulate on PE, then
            # f_scores row = kT^T @ qT in a single matvec -- no cross-partition
            # reshuffles needed.
            qT = qk_psum.tile([W, C], FP32, tag="qk")
            idx = 0
            for kt in range(KT):
                a3 = aggs[kt][:].rearrange("p (a c) -> p a c", c=PW)
                for tap in range(NTAP):
                    dy, dx = divmod(tap, 3)
                    nc.tensor.matmul(
                        qT[:], a3[:, dy, dx:dx + W],
                        wqT[kt][:, tap * C:(tap + 1) * C],
                        start=(idx == 0), stop=(idx == KT * NTAP - 1))
                    idx += 1
            kTp = misc_psum.tile([W, 1], FP32, tag="stp")
            idx = 0
            for kt in range(KT):
                a3 = aggs[kt][:].rearrange("p (a c) -> p a c", c=PW)
                for tap in range(NTAP):
                    dy, dx = divmod(tap, 3)
                    nc.tensor.matmul(
                        kTp[:], a3[:, dy, dx:dx + W], wks[kt][:, tap:tap + 1],
                        start=(idx == 0), stop=(idx == KT * NTAP - 1))
                    idx += 1
            qT_sb = small.tile([W, C], FP32, tag="qTsb")
            nc.vector.tensor_copy(qT_sb[:], qT[:])
            kT_sb = small.tile([W, 1], FP32, tag="kTsb")
            nc.vector.tensor_copy(kT_sb[:], kTp[:])
            fsrow = misc_psum.tile([1, C], FP32, tag="fsrow")
            nc.tensor.matmul(fsrow[:], kT_sb[:], qT_sb[:],
                             start=True, stop=True)
            mx = small.tile([1, 1], FP32, tag="mx")
            nc.vector.tensor_reduce(mx[:], fsrow[:], axis=AX.X, op=ALU.max)
            mxs = small.tile([1, 1], FP32, tag="mxs")
            nc.vector.tensor_scalar_mul(mxs[:], mx[:], -float(SCORE_SCALE))
            es = small.tile([1, C], FP32, tag="es")
            nc.scalar.activation(es[:], fsrow[:], ACTF.Exp,
                                 bias=mxs[:], scale=float(SCORE_SCALE))
            ssum = small.tile([1, 1], FP32, tag="ssum")
            nc.vector.tensor_reduce(ssum[:], es[:], axis=AX.X, op=ALU.add)
            rs = small.tile([1, 1], FP32, tag="rs")
            nc.vector.reciprocal(rs[:], ssum[:])
            srow = small.tile([1, C], FP32, tag="srow")
            nc.vector.tensor_scalar_mul(srow[:], es[:], rs[:])

            # scores back to [128,1] per mtile (K=1 matmul), fold in BN inv
            s1 = []
            for mt in range(MT):
                stp = misc_psum.tile([P, 1], FP32, tag="stp")
                nc.tensor.matmul(stp[:], srow[:, mt * P:(mt + 1) * P],
                                 ones_col[0:1, 0:1], start=True, stop=True)
                t = small.tile([P, 1], FP32, tag=f"s1{mt}")
                nc.vector.tensor_mul(t[:], stp[:], inv_t[mt][:])
                s1.append(t)

        if "conv" not in parts:
            continue
        # fv conv (18 accumulating matmuls per [128,512] tile) + fused
        # eviction: out = relu(fv*s1 + (x*inv + bias2))
        for mt in range(MT):
            for st in range(ST):
                y0 = st * ROWS_PER_ST
                pv = fv_psum.tile([P, NF], FP32, tag="fv")
                idx = 0
                for kt in range(KT):
                    x3 = xp[kt][:].rearrange("p (r c) -> p r c", c=PW)
                    for tap in range(NTAP):
                        dy, dx = divmod(tap, 3)
                        nc.tensor.matmul(
                            pv[:],
                            wvT[kt][:, tap * C + mt * P: tap * C + mt * P + P],
                            x3[:, y0 + dy:y0 + dy + ROWS_PER_ST, dx:dx + W],
                            start=(idx == 0), stop=(idx == KT * NTAP - 1))
                        idx += 1
                if "fullevict" in parts:
                    at = ev_pool.tile([P, NF], FP32, tag="A")
                    nc.scalar.activation(
                        at[:], xf[mt][:, st * NF:(st + 1) * NF], ACTF.Identity,
                        bias=bias2_t[mt][:], scale=inv_t[mt][:])
                    rt = ev_pool.tile([P, NF], FP32, tag="r")
                    nc.vector.scalar_tensor_tensor(
                        rt[:], pv[:], s1[mt][:], at[:],
                        op0=ALU.mult, op1=ALU.add)
                    o_t = ev_pool.tile([P, NF], FP32, tag="o")
                    nc.vector.tensor_scalar_max(o_t[:], rt[:], 0.0)
                else:
                    o_t = ev_pool.tile([P, NF], FP32, tag="o")
                    nc.vector.tensor_copy(o_t[:], pv[:])
                nc.sync.dma_start(
                    out[b, mt * P:(mt + 1) * P].rearrange(
                        "c h w -> c (h w)")[:, st * NF:(st + 1) * NF],
                    o_t[:])


def build_nc(repeat: int = 1, loop_n: int | None = None,
             parts=("scores", "conv", "fullevict"), hoist_prep: bool = False):
    nc = bacc.Bacc("TRN2", target_bir_lowering=False, debug=False,
                   num_devices=NCORES)
    x = nc.dram_tensor("x", [BPC, C, H, W], FP32, kind="ExternalInput").ap()
    wqT_d = nc.dram_tensor("wqT", [KT, P, NTAP * C], BF16,
                           kind="ExternalInput").ap()
    wvT_d = nc.dram_tensor("wvT", [KT, P, NTAP * C], BF16,
                           kind="ExternalInput").ap()
    wks_d = nc.dram_tensor("wks", [KT, P, NTAP], BF16,
                           kind="ExternalInput").ap()
    inv_d = nc.dram_tensor("inv", [C], FP32, kind="ExternalInput").ap()
    bias2_d = nc.dram_tensor("bias2", [C], FP32, kind="ExternalInput").ap()
    out = nc.dram_tensor("out", [BPC, C, H, W], FP32, kind="ExternalOutput").ap()
    with tile.TileContext(nc) as tc, ExitStack() as ctx:
        prep_state = None
        if hoist_prep:
            prep_state = _emit_prep(ctx, tc, nc, wqT_d, wvT_d, wks_d,
                                    inv_d, bias2_d)
        if loop_n is not None:
            with tc.For_i(0, loop_n, 1,
                          hint_engines=(mybir.EngineType.PE,)):
                with ExitStack() as rep_ctx:
                    _emit(rep_ctx, tc, nc, x, wqT_d, wvT_d, wks_d, inv_d,
                          bias2_d, out, parts=parts, prep_state=prep_state)
        else:
            for _ in range(repeat):
                with ExitStack() as rep_ctx:
                    _emit(rep_ctx, tc, nc, x, wqT_d, wvT_d, wks_d, inv_d,
                          bias2_d, out, parts=parts, prep_state=prep_state)
    nc.compile()
    return nc


_NC_CACHE = None


def _get_nc():
    global _NC_CACHE
    if _NC_CACHE is None:
        _NC_CACHE = build_nc()
    return _NC_CACHE


def make_in_maps(inputs: dict) -> list:
    import ml_dtypes
    f32 = lambda k: np.ascontiguousarray(np.asarray(inputs[k], np.float32))
    wq, wk, wv = f32("wq"), f32("wk"), f32("wv")
    gamma, beta = f32("gamma"), f32("beta")
    rmean, rvar = f32("running_mean"), f32("running_var")

    def tparts(w):
        # [o, i, dy, dx] -> per k-tile [i=128, (tap, o)] bf16
        a = w.reshape(C, KT, P, NTAP)              # o, kt, i, tap
        a = a.transpose(1, 2, 3, 0)                # kt, i, tap, o
        return np.ascontiguousarray(
            a.reshape(KT, P, NTAP * C).astype(ml_dtypes.bfloat16))

    wqT = tparts(wq)
    wvT = tparts(wv)
    wks = np.ascontiguousarray(
        wk.sum(axis=0).reshape(KT, P, NTAP).astype(ml_dtypes.bfloat16))
    inv = (gamma / np.sqrt(rvar + np.float32(EPS))).astype(np.float32)
    bias2 = (beta - rmean * inv).astype(np.float32)

    rep = {"wqT": wqT, "wvT": wvT, "wks": wks, "inv": inv, "bias2": bias2}
    xfull = np.ascontiguousarray(np.asarray(inputs["x"], dtype=np.float32))
    in_maps = []
    for c in range(NCORES):
        m = dict(rep)
        m["x"] = xfull[c * BPC:(c + 1) * BPC]
        in_maps.append(m)
    return in_maps


def kernel(**inputs) -> np.ndarray:
    import time
    nc = _get_nc()
    in_maps = make_in_maps(inputs)
    last_err = None
    for attempt in range(3):
        try:
            res = bass_utils.run_bass_kernel_spmd(
                nc, in_maps, core_ids=list(range(NCORES)))
            return np.concatenate(
                [res.results[c]["out"] for c in range(NCORES)],
                axis=0).astype(np.float32)
        except Exception as e:  # transient device/tunnel hiccups
            last_err = e
            time.sleep(3)
    raise last_err



# revision 2
# speedup vs baseline: 1.9496x; 1.9496x over previous
"""Trainium2 Bass kernel for nn_CSA_ConvBlock (conv-self-attention block), v2.

Reference math (B,C,H,W = 16,256,64,64):
  fq = conv3x3(x, wq); fk = conv3x3(x, wk); fv = conv3x3(x, wv)
  q_sum = fq.sum(H); k_sum = fk.sum(C,H)
  f_scores[b,c] = sum_w q_sum[b,c,w]*k_sum[b,w] / (sqrt(W)*H^2)
  scores = softmax_C(f_scores)
  out = relu(BN_eval(scores*fv + x))

v2 strategy vs v1 (bf16 shifted-window conv, 257us):
  * fv conv in fp8e4 with MatmulPerfMode.DoubleRow: the PE packs the
    channel pair (c, c+128) per cell, contracting K=256 per pass -- 9
    matmuls per [128,512] output tile instead of 18, at ~0.5 cyc/col.
    Moving operand is a host-prepadded pair-image [128, 2, 66, 80] fp8;
    each tap streams a 4D-AP window [128, 2, 8rows, 64cols].
  * Weights are host-scaled x64 into fp8 mid-range (w ~ N(0, 1/2304)
    would land in e4m3 subnormals); the 1/64 is folded into the
    per-channel score scale s1 = softmax * gamma/sqrt(var+eps) / 64.
    Since scores <= ~0.01, fp8 conv error is suppressed ~100x in the
    output (measured plan rel err 3e-3 vs 2e-2 budget).
  * x is uploaded bf16 for the residual/BN path and the q/k column sums
    (halves the HBM read vs fp32).
  * Column sums over H via a contiguous bf16 pairwise-halving tree on
    DVE (2 elem/cyc) instead of a strided 1 elem/cyc reduce.
  * Eviction: ACT computes x*inv+bias2 and the final relu; DVE does the
    single psum fused multiply-add. DMA evicts fp32.

Sharding: data-parallel over batch, 2 batches per core on 8 cores.
"""

import os
import sys
import numpy as np
from contextlib import ExitStack

if "/opt/trn_rl_repo" not in sys.path and not any(
    "trn_rl_repo" in p for p in sys.path
):
    sys.path.insert(0, "/opt/trn_rl_repo")

import concourse.bass as bass
import concourse.tile as tile
from concourse import bacc, mybir
from concourse import bass_utils

B, C, H, W = 16, 256, 64, 64
NCORES = 8
BPC = B // NCORES          # batches per core
P = 128                    # partitions
KT = C // P                # channel k-tiles (2)
MT = C // P                # channel m-tiles (2)
CP1 = C + 1                # q columns + k-sum column
NTAP = 9
ROWS = 8                   # rows per spatial tile
NF = ROWS * W              # 512 free elems per spatial tile
ST = (H * W) // NF         # 8 spatial tiles per (batch, mtile)
PH = H + 2                 # padded height 66
PWP = 80                   # padded row stride (16B-aligned pair blocks)
EPS = 1e-5
WS = 64.0                  # host-side fp8 weight scale
SCORE_SCALE = 1.0 / (np.sqrt(np.float32(W)) * (H * H))  # 1/32768

FP32 = mybir.dt.float32
BF16 = mybir.dt.bfloat16
FP8 = mybir.dt.float8e4
AX = mybir.AxisListType
ALU = mybir.AluOpType
ACTF = mybir.ActivationFunctionType
DR = mybir.MatmulPerfMode.DoubleRow


def _emit_prep(ctx, tc, nc, wv8_d, wqk_d, inv_d, invs_d, bias2_d):
    consts = ctx.enter_context(tc.tile_pool(name="consts", bufs=1))
    ones_col = consts.tile([P, 1], FP32, tag="ones")
    nc.vector.memset(ones_col[:], 1.0)

    inv_t, invs_t, bias2_t = [], [], []
    for mt in range(MT):
        iv = consts.tile([P, 1], FP32, tag=f"inv{mt}")
        nc.sync.dma_start(iv[:], inv_d[mt * P:(mt + 1) * P])
        inv_t.append(iv)
        ivs = consts.tile([P, 1], FP32, tag=f"invs{mt}")
        nc.sync.dma_start(ivs[:], invs_d[mt * P:(mt + 1) * P])
        invs_t.append(ivs)
        b2 = consts.tile([P, 1], FP32, tag=f"b2{mt}")
        nc.sync.dma_start(b2[:], bias2_d[mt * P:(mt + 1) * P])
        bias2_t.append(b2)

    wpool = ctx.enter_context(tc.tile_pool(name="w", bufs=1))
    wv_sb = [wpool.tile([P, NTAP, 2, P], FP8, tag=f"wv{mt}", name=f"wv{mt}")
             for mt in range(MT)]
    wqk_sb = [wpool.tile([P, NTAP, CP1], BF16, tag=f"wqk{kt}",
                         name=f"wqk{kt}") for kt in range(KT)]
    for mt in range(MT):
        nc.sync.dma_start(wv_sb[mt][:].rearrange("p a b c -> p (a b c)"),
                          wv8_d[mt])
    for kt in range(KT):
        nc.sync.dma_start(wqk_sb[kt][:].rearrange("p a b -> p (a b)"),
                          wqk_d[kt])
    return (consts, ones_col, inv_t, invs_t, bias2_t, wv_sb, wqk_sb)


def _emit_main(ctx, tc, nc, xb_d, xp8_d, out_d, prep, parts=('scores','evict','out')):
    consts, ones_col, inv_t, invs_t, bias2_t, wv_sb, wqk_sb = prep

    xb_pool = ctx.enter_context(tc.tile_pool(name="xb", bufs=2 * KT))
    xp_pool = ctx.enter_context(tc.tile_pool(name="xp", bufs=2))
    tree_pool = ctx.enter_context(tc.tile_pool(name="tree", bufs=2))
    agg_pool = ctx.enter_context(tc.tile_pool(name="agg", bufs=2 * KT))
    small = ctx.enter_context(tc.tile_pool(name="small", bufs=2))
    s1_pool = ctx.enter_context(tc.tile_pool(name="s1", bufs=2 * MT))
    ev_pool = ctx.enter_context(tc.tile_pool(name="ev", bufs=6))
    qk_psum = ctx.enter_context(
        tc.tile_pool(name="qk_psum", bufs=1, space="PSUM"))
    misc_psum = ctx.enter_context(
        tc.tile_pool(name="misc_psum", bufs=1, space="PSUM"))
    fv_psum = ctx.enter_context(
        tc.tile_pool(name="fv_psum", bufs=5, space="PSUM"))

    # ---- phase 1 (both batches): loads, column sums, scores ----
    xbt, xpt, s1 = [], [], []
    for b in range(BPC):
        xf = []
        for kt in range(KT):
            t = xb_pool.tile([P, H * W], BF16, tag="xb")
            nc.sync.dma_start(t[:], xb_d[b, kt * P:(kt + 1) * P])
            xf.append(t)
        xbt.append(xf)
        xp = xp_pool.tile([P, 2, PH, PWP], FP8, tag="xp")
        nc.sync.dma_start(xp[:].rearrange("p a b c -> p (a b c)"), xp8_d[b])
        xpt.append(xp)

        # column sums over H: contiguous pairwise-halving tree (bf16)
        aggs = []
        for kt in range(KT):
            cur = xf[kt][:]
            n = H * W
            while n > W:
                n //= 2
                dt = FP32 if n == W else BF16
                nxt = tree_pool.tile([P, n], dt, tag=f"tr{n}")
                nc.vector.tensor_add(nxt[:], cur[:, 0:n], cur[:, n:2 * n])
                cur = nxt
            cs = cur  # [P, W] fp32 column sums
            ag = agg_pool.tile([P, 3, H + 2], BF16, tag="agg")
            nc.vector.memset(ag[:, :, 0], 0.0)
            nc.vector.memset(ag[:, :, H + 1], 0.0)
            # dy=0 window is rows -1..H-2: colsum minus bottom row
            nc.vector.tensor_sub(ag[:, 0, 1:H + 1], cs[:],
                                 xf[kt][:, (H - 1) * W:H * W])
            nc.vector.tensor_copy(ag[:, 1, 1:H + 1], cs[:])
            # dy=2 window is rows 1..H: colsum minus top row
            nc.vector.tensor_sub(ag[:, 2, 1:H + 1], cs[:],
                                 xf[kt][:, 0:W])
            aggs.append(ag)

        if 'scores' not in parts:
            s1.append([invs_t[0], invs_t[1]])
            continue
        # q/k scores: qkp[w, 0:C] = q_sum^T, qkp[w, C] = k_sum
        qkp = qk_psum.tile([W, CP1], FP32, tag="qk")
        idx = 0
        for kt in range(KT):
            for tap in range(NTAP):
                dy, dx = divmod(tap, 3)
                nc.tensor.matmul(
                    qkp[:], aggs[kt][:, dy, dx:dx + W],
                    wqk_sb[kt][:, tap],
                    start=(idx == 0), stop=(idx == KT * NTAP - 1))
                idx += 1
        qk_sb = small.tile([W, CP1], FP32, tag="qksb")
        nc.vector.tensor_copy(qk_sb[:], qkp[:])
        fsrow = misc_psum.tile([1, C], FP32, tag="fsrow")
        nc.tensor.matmul(fsrow[:], qk_sb[:, C:C + 1], qk_sb[:, 0:C],
                         start=True, stop=True)
        mx = small.tile([1, 1], FP32, tag="mx")
        nc.vector.tensor_reduce(mx[:], fsrow[:], axis=AX.X, op=ALU.max)
        mxs = small.tile([1, 1], FP32, tag="mxs")
        nc.vector.tensor_scalar_mul(mxs[:], mx[:], -float(SCORE_SCALE))
        es = small.tile([1, C], FP32, tag="es")
        nc.scalar.activation(es[:], fsrow[:], ACTF.Exp,
                             bias=mxs[:], scale=float(SCORE_SCALE))
        ssum = small.tile([1, 1], FP32, tag="ssum")
        nc.vector.tensor_reduce(ssum[:], es[:], axis=AX.X, op=ALU.add)
        rs = small.tile([1, 1], FP32, tag="rs")
        nc.vector.reciprocal(rs[:], ssum[:])
        srow = small.tile([1, C], FP32, tag="srow")
        nc.vector.tensor_scalar_mul(srow[:], es[:], rs[:])

        # scores to [128,1] per mtile; fold in gamma/sqrt(var)/WS
        s1b = []
        for mt in range(MT):
            stp = misc_psum.tile([P, 1], FP32, tag="stp")
            nc.tensor.matmul(stp[:], srow[:, mt * P:(mt + 1) * P],
                             ones_col[0:1, 0:1], start=True, stop=True)
            t = s1_pool.tile([P, 1], FP32, tag=f"s1_{mt}")
            nc.vector.tensor_mul(t[:], stp[:], invs_t[mt][:])
            s1b.append(t)
        s1.append(s1b)

    # ---- phase 2 (both batches): fv conv + fused BN/relu eviction ----
    for b in range(BPC):
        for mt in range(MT):
            for st in range(ST):
                y0 = st * ROWS
                pv = fv_psum.tile([P, NF], FP32, tag="fv")
                for tap in range(NTAP):
                    dy, dx = divmod(tap, 3)
                    nc.tensor.matmul(
                        pv[:], wv_sb[mt][:, tap],
                        xpt[b][:, :, y0 + dy:y0 + dy + ROWS, dx:dx + W],
                        start=(tap == 0), stop=(tap == NTAP - 1),
                        perf_mode=DR)
                if 'evict' in parts:
                    at = ev_pool.tile([P, NF], FP32, tag="at")
                    nc.scalar.activation(
                        at[:], xbt[b][mt][:, st * NF:(st + 1) * NF],
                        ACTF.Identity, bias=bias2_t[mt][:], scale=inv_t[mt][:])
                    rt = ev_pool.tile([P, NF], FP32, tag="rt")
                    nc.vector.scalar_tensor_tensor(
                        rt[:], pv[:], s1[b][mt][:], at[:],
                        op0=ALU.mult, op1=ALU.add)
                    o_t = ev_pool.tile([P, NF], FP32, tag="o")
                    nc.scalar.activation(o_t[:], rt[:], ACTF.Relu)
                else:
                    o_t = ev_pool.tile([P, NF], FP32, tag="o")
                    nc.vector.tensor_copy(o_t[:], pv[:])
                if 'out' in parts or (b == BPC - 1 and mt == MT - 1
                                      and st == ST - 1):
                    nc.sync.dma_start(
                        out_d[b, mt * P:(mt + 1) * P][:, st * NF:(st + 1) * NF],
                        o_t[:])


def build_nc(repeat: int = 1, loop_n: int | None = None,
             parts=('scores', 'evict', 'out')):
    nc = bacc.Bacc("TRN2", target_bir_lowering=False, debug=False,
                   num_devices=NCORES)
    xb_d = nc.dram_tensor("xb", [BPC, C, H * W], BF16,
                          kind="ExternalInput").ap()
    xp8_d = nc.dram_tensor("xp8", [BPC, P, 2 * PH * PWP], FP8,
                           kind="ExternalInput").ap()
    wv8_d = nc.dram_tensor("wv8", [MT, P, NTAP * 2 * P], FP8,
                           kind="ExternalInput").ap()
    wqk_d = nc.dram_tensor("wqk", [KT, P, NTAP * CP1], BF16,
                           kind="ExternalInput").ap()
    inv_d = nc.dram_tensor("inv", [C], FP32, kind="ExternalInput").ap()
    invs_d = nc.dram_tensor("invs", [C], FP32, kind="ExternalInput").ap()
    bias2_d = nc.dram_tensor("bias2", [C], FP32, kind="ExternalInput").ap()
    out_d = nc.dram_tensor("out", [BPC, C, H * W], FP32,
                           kind="ExternalOutput").ap()
    with tile.TileContext(nc) as tc, ExitStack() as ctx:
        if loop_n is not None:
            with tc.For_i(0, loop_n, 1,
                          hint_engines=(mybir.EngineType.PE,)):
                with ExitStack() as rep_ctx:
                    prep = _emit_prep(rep_ctx, tc, nc, wv8_d, wqk_d, inv_d,
                                      invs_d, bias2_d)
                    _emit_main(rep_ctx, tc, nc, xb_d, xp8_d, out_d, prep,
                               parts=parts)
        else:
            for _ in range(repeat):
                with ExitStack() as rep_ctx:
                    prep = _emit_prep(rep_ctx, tc, nc, wv8_d, wqk_d, inv_d,
                                      invs_d, bias2_d)
                    _emit_main(rep_ctx, tc, nc, xb_d, xp8_d, out_d, prep,
                               parts=parts)
    nc.compile()
    return nc


_NC_CACHE = None


def _get_nc():
    global _NC_CACHE
    if _NC_CACHE is None:
        _NC_CACHE = build_nc()
    return _NC_CACHE


def make_in_maps(inputs: dict) -> list:
    import ml_dtypes
    f32 = lambda k: np.ascontiguousarray(np.asarray(inputs[k], np.float32))
    wq, wk, wv = f32("wq"), f32("wk"), f32("wv")
    gamma, beta = f32("gamma"), f32("beta")
    rmean, rvar = f32("running_mean"), f32("running_var")

    # wv8[mt, p, tap, k, o] = wv[mt*128+o, k*128+p, dy, dx] * WS  (fp8)
    a = (wv * WS).reshape(MT, P, KT, P, NTAP)       # mt, o, k, p, tap
    a = a.transpose(0, 3, 4, 2, 1)                  # mt, p, tap, k, o
    wv8 = np.ascontiguousarray(
        a.reshape(MT, P, NTAP * 2 * P).astype(ml_dtypes.float8_e4m3))

    # wqk[kt, p, tap, 0:C] = wq[:, kt*128+p, dy, dx]^T; [..., C] = wk colsum
    q = wq.reshape(C, KT, P, NTAP).transpose(1, 2, 3, 0)   # kt, p, tap, o
    ks = wk.sum(axis=0).reshape(KT, P, NTAP)[..., None]    # kt, p, tap, 1
    wqk = np.ascontiguousarray(
        np.concatenate([q, ks], axis=-1)
        .reshape(KT, P, NTAP * CP1).astype(ml_dtypes.bfloat16))

    inv = (gamma / np.sqrt(rvar + np.float32(EPS))).astype(np.float32)
    invs = (inv / np.float32(WS)).astype(np.float32)
    bias2 = (beta - rmean * inv).astype(np.float32)

    xfull = np.asarray(inputs["x"], dtype=np.float32)
    xb_all = xfull.reshape(B, C, H * W).astype(ml_dtypes.bfloat16)
    # padded fp8 pair-image: [B, p, k, PH, PWP], interior rows/cols at +1
    x8 = xfull.reshape(B, KT, P, H, W).astype(ml_dtypes.float8_e4m3)
    xp8_all = np.zeros((B, P, KT, PH, PWP), ml_dtypes.float8_e4m3)
    xp8_all[:, :, :, 1:H + 1, 1:W + 1] = x8.transpose(0, 2, 1, 3, 4)
    xp8_all = xp8_all.reshape(B, P, 2 * PH * PWP)

    rep = {"wv8": wv8, "wqk": wqk, "inv": inv, "invs": invs, "bias2": bias2}
    in_maps = []
    for c in range(NCORES):
        m = dict(rep)
        m["xb"] = np.ascontiguousarray(xb_all[c * BPC:(c + 1) * BPC])
        m["xp8"] = np.ascontiguousarray(xp8_all[c * BPC:(c + 1) * BPC])
        in_maps.append(m)
    return in_maps


def kernel(**inputs) -> np.ndarray:
    import time
    nc = _get_nc()
    in_maps = make_in_maps(inputs)
    last_err = None
    for attempt in range(3):
        try:
            res = bass_utils.run_bass_kernel_spmd(
                nc, in_maps, core_ids=list(range(NCORES)))
            return np.concatenate(
                [res.results[c]["out"] for c in range(NCORES)],
                axis=0).astype(np.float32).reshape(B, C, H, W)
        except Exception as e:  # transient device/tunnel hiccups
            last_err = e
            time.sleep(3)
    raise last_err
